# revision 1
# baseline (speedup 1.0000x reference)
"""ChebConv K=2 (L_hat = -D^-1/2 A D^-1/2) distributed over 8 NeuronCores.

Sharding (per spec hint): nodes 12500/core; edges partitioned by destination
shard. Two SPMD launches:

  L1 (row-sharded edges): deg = segment_sum(w, row) via a padded per-node
     weight table + one batched free-dim reduce; dinv = deg>0 ? rsqrt(deg) : 0;
     Z = dinv ⊙ (x @ W1) in fp16; U = x @ W0 + b (bias via augmented ones row).
  host: concatenates Z shards -> Zfull (layout only, no arithmetic) and builds
     per-core one-hot routing matrices S (compile-schedule-static layout,
     per-core values: S[k, inst, dst] = -w_e for edge e at gather slot k of
     instance inst with local dest slot dst).
  L2 (dest-sharded edges): per super-group of 8 dest groups, gather Z rows of
     edge sources (dma_gather fp16, int16 indices bucketed by source range,
     4 SWDGE queues), load S slabs via HWDGE, accumulate S^T @ Zg in PSUM over
     instance tiles, out = dinv ⊙ psum + u.

Identity: out = x@W0 + b + dinv_col ⊙ Σ_e 1[col=n](-w_e)(dinv⊙(x@W1))[row_e]
        = x@W0 + Tx1@W1 + b with Tx1 = segment_sum(norm * x[row], col).

Edge schedule is equalized across cores (segment sizes = max over cores) so
one SPMD kernel serves all 8 cores; per-core shortfall gathers row 0 with a
zero S column. Tiles straddling group boundaries run once per touched group;
other-group edges have zero S entries, so no masking is needed.
"""
import sys

if "/opt/trn_rl_repo" not in sys.path:
    sys.path.insert(0, "/opt/trn_rl_repo")

import numpy as np

import concourse.bass as bass
import concourse.bacc as bacc
import concourse.mybir as mybir
import concourse.tile as tile
from concourse.bass_utils import run_bass_kernel_spmd

P = 128
D = 64
N_NODES = 100000
N_CORES = 8
NSH = N_NODES // N_CORES            # 12500 nodes per shard
NG = (NSH + P - 1) // P             # 98 groups per shard
SG_GROUPS = 8                       # groups per gather super-call
NSG = (NG + SG_GROUPS - 1) // SG_GROUPS
BUCKET = 25000                      # z-table bucket rows (int16-addressable)
NBUCKETS = (N_NODES + BUCKET - 1) // BUCKET

F32 = mybir.dt.float32
F16 = mybir.dt.float16
I16 = mybir.dt.int16

_cache = {}
LAST_STATS = {}


# ----------------------------------------------------------------- L1 kernel
def build_l1(kd):
    nc = bacc.Bacc("TRN2", target_bir_lowering=False, debug=False,
                   num_devices=N_CORES)
    xta_d = nc.dram_tensor("xta", [D + 1, NSH], F16, kind="ExternalInput")
    wpad_d = nc.dram_tensor("wpad", [P, NG * kd], F16, kind="ExternalInput")
    w0a_d = nc.dram_tensor("w0a", [D + 1, D], F16, kind="ExternalInput")
    w1_d = nc.dram_tensor("w1", [D, D], F16, kind="ExternalInput")
    z_d = nc.dram_tensor("z", [NSH, D], F16, kind="ExternalOutput")
    u_d = nc.dram_tensor("u", [NSH, D], F32, kind="ExternalOutput")
    dinv_d = nc.dram_tensor("dinv", [P, NG], F32, kind="ExternalOutput")

    with tile.TileContext(nc) as tc:
        with (
            tc.tile_pool(name="const", bufs=1) as cpool,
            tc.tile_pool(name="sbuf", bufs=3) as pool,
            tc.tile_pool(name="psum", bufs=2, space="PSUM") as psum_pool,
        ):
            w0a_t = cpool.tile([D + 1, D], F16)
            nc.sync.dma_start(w0a_t[:], w0a_d[:, :])
            w1_t = cpool.tile([D, D], F16)
            nc.sync.dma_start(w1_t[:], w1_d[:, :])
            wbig = cpool.tile([P, NG, kd], F16)
            nc.sync.dma_start(wbig[:], wpad_d[:, :])
            xta_t = cpool.tile([D + 1, NSH], F16)
            nc.sync.dma_start(xta_t[:], xta_d[:, :])

            deg_t = cpool.tile([P, NG], F32)
            nc.vector.reduce_sum(deg_t[:], wbig[:], axis=mybir.AxisListType.X)
            m_t = cpool.tile([P, NG], F32)
            nc.vector.tensor_scalar_max(m_t[:], deg_t[:], 1e-30)
            s_t = cpool.tile([P, NG], F32)
            nc.scalar.activation(s_t[:], m_t[:], mybir.ActivationFunctionType.Sqrt)
            r_t = cpool.tile([P, NG], F32)
            nc.vector.reciprocal(r_t[:], s_t[:])
            mask_t = cpool.tile([P, NG], F32)
            nc.vector.tensor_scalar(
                out=mask_t[:], in0=deg_t[:], scalar1=0.0, scalar2=None,
                op0=mybir.AluOpType.is_gt,
            )
            dinv_t = cpool.tile([P, NG], F32)
            nc.vector.tensor_mul(dinv_t[:], r_t[:], mask_t[:])
            nc.sync.dma_start(dinv_d[:, :], dinv_t[:])

            for sg in range(NSG):
                g0 = sg * SG_GROUPS
                g1 = min(g0 + SG_GROUPS, NG)
                ng = g1 - g0
                z_sg = pool.tile([P, SG_GROUPS, D], F16, tag="z")
                u_sg = pool.tile([P, SG_GROUPS, D], F32, tag="u")
                for g in range(g0, g1):
                    n0 = g * P
                    n1 = min(n0 + P, NSH)
                    np_ = n1 - n0
                    v_p = psum_pool.tile([P, D], F32, tag="vp",
                                         space="PSUM")
                    nc.tensor.matmul(out=v_p[:np_], lhsT=xta_t[:D, n0:n1],
                                     rhs=w1_t[:], start=True, stop=True)
                    da = dinv_t[:np_, g:g + 1]
                    din = bass.AP(da.tensor, da.offset, [da.ap[0], [0, D]])
                    nc.vector.tensor_tensor(
                        out=z_sg[:np_, g - g0, :], in0=v_p[:np_], in1=din,
                        op=mybir.AluOpType.mult)
                    u_p = psum_pool.tile([P, D], F32, tag="up",
                                         space="PSUM")
                    nc.tensor.matmul(out=u_p[:np_], lhsT=xta_t[:, n0:n1],
                                     rhs=w0a_t[:], start=True, stop=True)
                    nc.vector.tensor_copy(u_sg[:np_, g - g0, :], u_p[:np_])
                n0 = g0 * P
                n1 = min(g1 * P, NSH)
                nfull = (n1 - n0) // P        # whole-128 groups in this sg
                if nfull:
                    za = z_d[n0:n0 + nfull * P, :]
                    zap = bass.AP(za.tensor, za.offset,
                                  [[D, P], [P * D, nfull], [1, D]])
                    nc.sync.dma_start(zap, z_sg[:, :nfull, :])
                    ua = u_d[n0:n0 + nfull * P, :]
                    uap = bass.AP(ua.tensor, ua.offset,
                                  [[D, P], [P * D, nfull], [1, D]])
                    nc.sync.dma_start(uap, u_sg[:, :nfull, :])
                rem = (n1 - n0) - nfull * P   # trailing partial group rows
                if rem:
                    nc.sync.dma_start(z_d[n0 + nfull * P:n1, :],
                                      z_sg[:rem, nfull, :])
                    nc.sync.dma_start(u_d[n0 + nfull * P:n1, :],
                                      u_sg[:rem, nfull, :])
    nc.compile()
    return nc


# ----------------------------------------------------------------- L2 kernel
def build_l2(sched):
    """sched: static schedule, same for all cores (see _prep_l2)."""
    calls, ginsts, sg_tiles, sg_insts, tot16, tot_inst = sched
    max_sg_tiles = max(sg_tiles)
    max_sg_insts = max(e - s for s, e in sg_insts)

    nc = bacc.Bacc("TRN2", target_bir_lowering=False, debug=False,
                   num_devices=N_CORES, num_swdge_queues=4)
    z_d = nc.dram_tensor("zfull", [N_NODES, 2 * D], F16, kind="ExternalInput")
    u_d = nc.dram_tensor("u", [NSH, D], F32, kind="ExternalInput")
    dinv_d = nc.dram_tensor("dinv", [P, NG], F32, kind="ExternalInput")
    gidx_d = nc.dram_tensor("gidx", [P, tot16], I16, kind="ExternalInput")
    s_d = nc.dram_tensor("smat", [P, tot_inst * P], F16, kind="ExternalInput")
    out_d = nc.dram_tensor("out", [NSH, D], F32, kind="ExternalOutput")

    with tile.TileContext(nc) as tc:
        with (
            tc.tile_pool(name="const", bufs=1) as cpool,
            tc.tile_pool(name="gb", bufs=2) as gpool,
            tc.tile_pool(name="sb", bufs=2) as spool,
            tc.tile_pool(name="io", bufs=2) as iopool,
            tc.tile_pool(name="psum", bufs=1, space="PSUM") as psum_pool,
        ):
            dinv_t = cpool.tile([P, NG], F32)
            nc.sync.dma_start(dinv_t[:], dinv_d[:, :])
            gbufs = [cpool.tile([P, max_sg_tiles, 2 * D], F16, name=f"gbuf{i}")
                     for i in range(2)]

            for sg in range(NSG):
                g0 = sg * SG_GROUPS
                g1 = min(g0 + SG_GROUPS, NG)
                ng = g1 - g0
                gbuf = gbufs[sg % 2]
                sg_calls = [c for c in calls if c[0] == sg]
                i16_lo = min(c[3] for c in sg_calls)
                i16_hi = max(c[3] + c[1] // 16 for c in sg_calls)
                idx_t = iopool.tile([P, i16_hi - i16_lo], I16, tag="idx")
                nc.sync.dma_start(idx_t[:], gidx_d[:, i16_lo:i16_hi])
                for (csg, num_idxs, valid, i16_off, tile_off, b) in sg_calls:
                    b0 = b * BUCKET
                    b1 = min(b0 + BUCKET, N_NODES)
                    nc.gpsimd.dma_gather(
                        out_ap=gbuf[:, tile_off:tile_off + num_idxs // P, :],
                        in_ap=z_d[b0:b1, :],
                        idxs_ap=idx_t[:, i16_off - i16_lo:
                                      i16_off - i16_lo + num_idxs // 16],
                        num_idxs=num_idxs,
                        num_idxs_reg=num_idxs,
                        elem_size=2 * D,
                        single_packet=False,
                        queue_num=b % 4,
                    )
                # S slab for this sg
                ilo, ihi = sg_insts[sg]
                s_t = spool.tile([P, max_sg_insts, P], F16, tag="s")
                nc.sync.dma_start(s_t[:, :ihi - ilo, :],
                                  s_d[:, ilo * P:ihi * P])
                # u slab
                n0 = g0 * P
                n1 = min(g1 * P, NSH)
                nfull = (n1 - n0) // P
                rem = (n1 - n0) - nfull * P
                u_sg = iopool.tile([P, SG_GROUPS, D], F32, tag="u")
                if nfull:
                    ua = u_d[n0:n0 + nfull * P, :]
                    uap = bass.AP(ua.tensor, ua.offset,
                                  [[D, P], [P * D, nfull], [1, D]])
                    nc.sync.dma_start(u_sg[:, :nfull, :], uap)
                if rem:
                    nc.sync.dma_start(u_sg[:rem, nfull, :],
                                      u_d[n0 + nfull * P:n1, :])
                out_sg = iopool.tile([P, SG_GROUPS, D], F32, tag="o")

                for g in range(g0, g1):
                    insts = ginsts[g]     # list of (inst_id, tile_in_sg)
                    kb = len(insts)
                    nlast = min(P, NSH - g * P)
                    psum = psum_pool.tile([P, D], F32, tag=f"acc{g % 8}",
                                          space="PSUM")
                    for j, (inst_id, t) in enumerate(insts):
                        nc.tensor.matmul(
                            out=psum[:],
                            lhsT=s_t[:, inst_id - ilo, :],
                            rhs=gbuf[:, t, 0:D],
                            start=(j == 0),
                            stop=(j == kb - 1),
                        )
                    da = dinv_t[:nlast, g:g + 1]
                    din = bass.AP(da.tensor, da.offset, [da.ap[0], [0, D]])
                    o_sl = out_sg[:nlast, g - g0, :]
                    nc.vector.tensor_tensor(out=o_sl, in0=psum[:nlast],
                                            in1=din, op=mybir.AluOpType.mult)
                    nc.vector.tensor_add(o_sl, o_sl,
                                         u_sg[:nlast, g - g0, :])
                if nfull:
                    oa = out_d[n0:n0 + nfull * P, :]
                    oap = bass.AP(oa.tensor, oa.offset,
                                  [[D, P], [P * D, nfull], [1, D]])
                    nc.sync.dma_start(oap, out_sg[:, :nfull, :])
                if rem:
                    nc.sync.dma_start(out_d[n0 + nfull * P:n1, :],
                                      out_sg[:rem, nfull, :])
    nc.compile()
    return nc


# ------------------------------------------------------------- host prep
def _prep_l1(row, w):
    """Per-core padded weight tables. Returns (kd, list of [P, NG*kd] f16)."""
    core = row // NSH
    data = []
    kd = 4
    for c in range(N_CORES):
        sel = core == c
        r_loc = (row[sel] - c * NSH).astype(np.int64)
        w_c = w[sel]
        counts = np.bincount(r_loc, minlength=NSH)
        kd = max(kd, int(counts.max()))
        data.append((r_loc, w_c, counts))
    kd = ((kd + 3) // 4) * 4
    out = []
    for r_loc, w_c, counts in data:
        offs = np.cumsum(counts) - counts
        order = np.argsort(r_loc, kind="stable")
        r_s = r_loc[order]
        w_s = w_c[order]
        k = np.arange(len(r_s)) - offs[r_s]
        wpad = np.zeros((NG * P, kd), np.float16)
        wpad[r_s, k] = w_s
        wbig = wpad.reshape(NG, P, kd).transpose(1, 0, 2).reshape(P, NG * kd)
        out.append(np.ascontiguousarray(wbig))
    return kd, out


def _prep_l2(row, col, w):
    """Core-equalized L2 schedule + per-core gidx/S arrays."""
    core = col // NSH
    percore = []
    counts = np.zeros((N_CORES, NG, NBUCKETS), np.int64)
    for c in range(N_CORES):
        sel = core == c
        rows = row[sel]
        col_loc = col[sel] - c * NSH
        w_c = w[sel]
        g = col_loc // P
        slot = col_loc % P
        b = rows // BUCKET
        rel = rows % BUCKET
        order = np.lexsort((rel, b, g))
        percore.append((g[order], slot[order], b[order], rel[order], w_c[order]))
        cnt = np.bincount(g * NBUCKETS + b, minlength=NG * NBUCKETS)
        counts[c] = cnt.reshape(NG, NBUCKETS)
    smax = counts.max(axis=0)          # [NG, NBUCKETS] equalized segment sizes

    # --- static schedule: calls per (sg, bucket) ---
    calls = []        # (sg, num_idxs, valid, i16_off, tile_off, bucket)
    seg_pos = np.zeros((NG, NBUCKETS), np.int64)   # seg start within call
    seg_call = np.zeros((NG, NBUCKETS), np.int64)  # call id of segment
    sg_tiles = []
    i16_off = 0
    for sg in range(NSG):
        g0, g1 = sg * SG_GROUPS, min((sg + 1) * SG_GROUPS, NG)
        toff = 0
        for b in range(NBUCKETS):
            valid = int(smax[g0:g1, b].sum())
            if valid == 0:
                continue
            num_idxs = -(-valid // P) * P
            pos = 0
            for g in range(g0, g1):
                seg_pos[g, b] = pos
                seg_call[g, b] = len(calls)
                pos += int(smax[g, b])
            calls.append((sg, num_idxs, valid, i16_off, toff, b))
            i16_off += num_idxs // 16
            toff += num_idxs // P
        sg_tiles.append(toff)
    tot16 = i16_off

    # --- instances: per group, tiles it draws from (may straddle) ---
    ginsts = []            # per g: tuple of (inst_id, tile_in_sg)
    inst_of = {}           # (g, tile_in_sg) -> inst_id
    sg_insts = []          # per sg: (inst_lo, inst_hi)
    inst_id = 0
    for sg in range(NSG):
        g0, g1 = sg * SG_GROUPS, min((sg + 1) * SG_GROUPS, NG)
        lo = inst_id
        for g in range(g0, g1):
            lst = []
            for b in range(NBUCKETS):
                s = int(smax[g, b])
                if s == 0:
                    continue
                cid = seg_call[g, b]
                tile_off = calls[cid][4]
                a = int(seg_pos[g, b])
                t0 = a // P
                t1 = -(-(a + s) // P)
                for t in range(t0, t1):
                    key = (g, tile_off + t)
                    if key not in inst_of:
                        inst_of[key] = inst_id
                        inst_id += 1
                    lst.append((inst_of[key], tile_off + t))
            # dedupe (t seen via two buckets straddling) keeping order
            seen = set()
            lst2 = []
            for it in lst:
                if it not in seen:
                    seen.add(it)
                    lst2.append(it)
            ginsts.append(tuple(sorted(lst2)))
        sg_insts.append((lo, inst_id))
    tot_inst = inst_id

    sched = (tuple(calls), tuple(ginsts), tuple(sg_tiles), tuple(sg_insts),
             tot16, tot_inst)

    # --- per-core arrays: gidx + S ---
    arrays = []
    for c in range(N_CORES):
        g_e, slot_e, b_e, rel_e, w_e = percore[c]
        cnt = counts[c]
        seg_id = g_e * NBUCKETS + b_e
        cnt_flat = cnt.reshape(-1)
        offs_e = np.cumsum(cnt_flat) - cnt_flat
        pos_in_seg = np.arange(len(g_e)) - offs_e[seg_id]
        abs_pos = seg_pos.reshape(-1)[seg_id] + pos_in_seg
        call_of_e = seg_call.reshape(-1)[seg_id]

        gidx = np.zeros((P, tot16), np.int16)
        for cid, (sg, num_idxs, valid, i16o, tile_off, b) in enumerate(calls):
            sel = call_of_e == cid
            seq = np.zeros(num_idxs, np.int64)
            seq[abs_pos[sel]] = rel_e[sel]
            wr = seq.reshape(num_idxs // 16, 16).T.astype(np.int16)
            gidx[:, i16o:i16o + num_idxs // 16] = np.tile(wr, (8, 1))

        # S fill: edge -> (k, inst*P + slot)
        tile_in_sg = np.zeros(len(g_e), np.int64)
        for cid, (sg, num_idxs, valid, i16o, tile_off, b) in enumerate(calls):
            sel = call_of_e == cid
            tile_in_sg[sel] = tile_off + abs_pos[sel] // P
        k_e = abs_pos % P
        # map (g, tile_in_sg) -> inst via vectorized lookup
        keys = g_e * 100000 + tile_in_sg
        lut_keys = np.array([g * 100000 + t for (g, t) in inst_of.keys()],
                            np.int64)
        lut_vals = np.array(list(inst_of.values()), np.int64)
        order_l = np.argsort(lut_keys)
        pos_l = np.searchsorted(lut_keys[order_l], keys)
        inst_e = lut_vals[order_l][pos_l]
        smat = np.zeros((P, tot_inst * P), np.float16)
        smat[k_e, inst_e * P + slot_e] = (-w_e).astype(np.float16)
        arrays.append({"gidx": gidx, "smat": smat})
    return sched, arrays


# ------------------------------------------------------------------ kernel()
def kernel(x, edge_index, edge_weight, W0, W1, b):
    global LAST_STATS
    x = np.asarray(x, np.float32)
    edge_index = np.asarray(edge_index)
    w = np.asarray(edge_weight, np.float32)
    W0 = np.asarray(W0, np.float32)
    W1 = np.asarray(W1, np.float32)
    b = np.asarray(b, np.float32)
    row = edge_index[0].astype(np.int64)
    col = edge_index[1].astype(np.int64)

    kd, wpads = _prep_l1(row, w)
    sched, l2arr = _prep_l2(row, col, w)
    sched_key = (sched[0], sched[2], sched[3], sched[4], sched[5])

    if ("l1", kd) not in _cache:
        _cache[("l1", kd)] = build_l1(kd)
    nc1 = _cache[("l1", kd)]
    if ("l2", sched_key) not in _cache:
        _cache[("l2", sched_key)] = build_l2(sched)
    nc2 = _cache[("l2", sched_key)]

    w0a = np.concatenate([W0, b.reshape(1, D)], axis=0).astype(np.float16)
    w1h = W1.astype(np.float16)
    in1 = []
    for c in range(N_CORES):
        xs = x[c * NSH:(c + 1) * NSH]
        xta = np.concatenate([xs.T, np.ones((1, NSH), np.float32)],
                             axis=0).astype(np.float16)
        in1.append({"xta": np.ascontiguousarray(xta), "wpad": wpads[c],
                    "w0a": w0a, "w1": w1h})
    res1 = run_bass_kernel_spmd(nc1, in1, core_ids=list(range(N_CORES)))
    zfull = np.concatenate([res1.results[c]["z"] for c in range(N_CORES)],
                           axis=0)
    zfull2 = np.ascontiguousarray(np.concatenate([zfull, zfull], axis=1))
    in2 = [
        {"zfull": zfull2, "u": res1.results[c]["u"],
         "dinv": res1.results[c]["dinv"],
         "gidx": l2arr[c]["gidx"], "smat": l2arr[c]["smat"]}
        for c in range(N_CORES)
    ]
    res2 = run_bass_kernel_spmd(nc2, in2, core_ids=list(range(N_CORES)))
    out = np.concatenate([res2.results[c]["out"] for c in range(N_CORES)],
                         axis=0)
    LAST_STATS = {
        "l1_exec_ns": res1.exec_time_ns,
        "l2_exec_ns": res2.exec_time_ns,
        "descs": sum(c[2] for c in sched[0]),
        "insts": sched[5],
    }
    return out.astype(np.float32)

